# revision 12
# baseline (speedup 1.0000x reference)
"""Trainium2 Bass kernel for nn_GatedJunction (gated multi-branch junction).

Math (per batch element b):
    m_y  = mean_hw(y[b])                     # [C]
    m_xk = mean_hw(x_k[b])                   # [C] for k=0..3
    feats = concat(m_y, m_x0..m_x3)          # [5C] = [1280]
    h  = relu(bn(feats @ conv1_w.T))         # [32]
    w  = h @ conv2_w.T + conv2_b             # [1280] -> [5, 256]
    w1 = sigmoid(w[0])                       # self gate  [256]
    w2 = softmax_k(w[1:])                    # branch gates [4, 256]
    out[b] = y[b]*w1[:,None,None] + sum_k w2[k][:,None,None]*x_k[b]

Sharding: data-parallel over batch. 8 cores x 4 batch elements each.
Params are tiny, folded on the host (BN affine, weight transposes) and
replicated to every core.

fp16 end-to-end on the HBM side: inputs are cast to fp16 on the host
(halves the DMA traffic of this memory-bound problem; per-core traffic
24 MiB -> 12 MiB) and the output is stored fp16 and widened on the
host. fp16 keeps ~6e-4 relative error, far inside the 2e-2 gate.

On-core layout: channel-on-partition; batches are loaded in PAIRS (one
1 MiB DMA per tensor per pair -> 10 loads + 2 stores per 4-batch pass;
fewer, larger DMAs measured faster than per-batch 512 KiB ones).
Engine split (HW evidence: PE sequencer dispatch is expensive, so PE
only runs the tiny gate MLP; bulk elementwise is DVE with ACT overflow):
  - DVE:  channel sums for y/x0/x1 (tensor_scalar accum, fp16 4x mode),
          softmax bits, and the 4-step scalar_tensor_tensor chains
          acc += w2k * xk for both channel halves.
  - ACT:  channel sums for x2/x3 (activation accum), BN+ReLU,
          sigmoid/exp, and the chain starts acc = y*w1 (Copy w/ scale).
  - PE:   the 1280->32->1280 gate MLP in fp32 (20 small matmuls).
  - Stores ride the ACT HWDGE ring so the SP ring carries only loads.
"""

import sys

for _p in ("/root/.axon_site/_ro/trn_rl_repo", "/opt/trn_rl_repo"):
    if _p not in sys.path:
        sys.path.append(_p)

from contextlib import ExitStack

import numpy as np

import concourse.bass as bass
import concourse.tile as tile
from concourse import mybir
from concourse.bass_utils import run_bass_kernel_spmd

# Problem constants (hardcoded from the spec).
B, K, C, H, W = 32, 4, 256, 32, 32
MID = 32
EPS = 1e-5
HW = H * W          # 1024
N_CORES = 8
B_LOC = B // N_CORES  # 4
NP = B_LOC // 2     # batch pairs per core
NT = K + 1          # 5 tensors: y, x0..x3
FEAT = NT * C       # 1280
NCH = FEAT // 128   # 10 feature chunks of 128
CH = C // 128       # 2 channel chunks per tensor

FP32 = mybir.dt.float32
FP16 = mybir.dt.float16
ALU = mybir.AluOpType
AF = mybir.ActivationFunctionType


def _split_waits(nc: bass.Bass) -> None:
    """This toolchain's walrus accepts only ONE sync-wait per instruction
    (setupSyncWait: 'Too many sync wait commands') while Tile emits several.
    Hoist all-but-one wait onto standalone EventSemaphore instructions
    placed immediately before, on the same engine — semantically identical
    (sequencer stalls at each wait in order)."""
    for f in nc.m.functions:
        for blk in f.blocks:
            insts = list(blk.instructions)
            out, changed = [], False
            for inst in insts:
                si = inst.sync_info
                if si is not None and len(si.on_wait) > 1:
                    waits = list(si.on_wait)
                    for i, w in enumerate(waits[:-1]):
                        ev = mybir.InstEventSemaphore(
                            name=f"{inst.name}-sw{i}", ins=[], outs=[]
                        )
                        ev.engine = inst.engine
                        ev.sync_info = mybir.SyncInfo(on_wait=[w], on_update=[])
                        out.append(ev)
                    si.on_wait = [waits[-1]]
                    changed = True
                out.append(inst)
            if changed:
                blk.instructions = out


def build_program(debug: bool = False, repeat: int = 1) -> bass.Bass:
    """Emit the single-core SPMD program (same program, per-core data).

    repeat > 1 re-runs the whole batch loop (idempotent) — used only for
    launch-overhead-cancelling timing in test.py.
    """
    nc = bass.Bass()

    d_in = [
        nc.declare_dram_parameter(nm, [B_LOC, 128, CH, HW], FP16, isOutput=False)
        for nm in ("y", "x0", "x1", "x2", "x3")
    ]
    # Pre-transposed / pre-folded params (host side):
    #   w1T[p, j, m] = conv1_w[m, 128j+p]
    #   w2T[m, j, p] = conv2_w[128j+p, m]
    #   c2bT[p, j]   = conv2_b[128j+p]
    #   scale_eff = (gamma/sqrt(var+eps))/HW, bias_eff = beta - mean*gamma/sqrt(var+eps)
    d_w1T = nc.declare_dram_parameter("w1T", [128, NCH, MID], FP32, isOutput=False)
    d_w2T = nc.declare_dram_parameter("w2T", [MID, NCH, 128], FP32, isOutput=False)
    d_c2bT = nc.declare_dram_parameter("c2bT", [128, NCH, 2], FP32, isOutput=False)
    d_scale = nc.declare_dram_parameter("scale_eff", [MID, 1], FP32, isOutput=False)
    d_bias = nc.declare_dram_parameter("bias_eff", [MID, 1], FP32, isOutput=False)
    d_out = nc.declare_dram_parameter("out", [B_LOC, 128, CH, HW], FP16, isOutput=True)

    with tile.TileContext(nc) as tc, ExitStack() as ctx:
        cpool = ctx.enter_context(tc.tile_pool(name="cpool", bufs=1))
        ppool = ctx.enter_context(tc.tile_pool(name="ppool", bufs=1, space="PSUM"))
        dpool = ctx.enter_context(tc.tile_pool(name="dpool", bufs=2))
        spool = ctx.enter_context(tc.tile_pool(name="spool", bufs=2))

        # ---------------- parameter loads (once) ----------------
        w1T = cpool.tile([128, NCH, MID], FP32, name="w1T", tag="w1T")
        nc.scalar.dma_start(out=w1T[:], in_=d_w1T[:])
        w2T = cpool.tile([MID, NCH, 128], FP32, name="w2T", tag="w2T")
        nc.scalar.dma_start(out=w2T[:], in_=d_w2T[:])
        c2bT = cpool.tile([128, NCH, 2], FP32, name="c2bT", tag="c2bT")
        nc.scalar.dma_start(out=c2bT[:], in_=d_c2bT[:])
        scale_eff = cpool.tile([MID, 1], FP32, name="scale_eff", tag="scale_eff")
        nc.scalar.dma_start(out=scale_eff[:], in_=d_scale[:])
        bias_eff = cpool.tile([MID, 1], FP32, name="bias_eff", tag="bias_eff")
        nc.scalar.dma_start(out=bias_eff[:], in_=d_bias[:])

        # ---------------- main loop over local batch pairs ----------------
        # Software-pipelined: section p computes sums/MLP/gates for pair p,
        # then runs pass 2 + store for pair p-1 (whose gates are ready), so
        # no engine stream stalls waiting on the same batch's gate chain.
        prev = None  # (tiles2, acc2, gates[bb] = (gat_s, gat_n), p)

        def emit_pass2(state):
            tiles2, acc2, gates, pp = state
            for bb in range(2):
                gat_s, gat_n = gates[0]
                terms = []
                for t in range(NT):
                    tm = spool.tile(
                        [128, CH, HW], FP16, name=f"tm{t}", tag=f"tm{t}", bufs=2
                    )
                    for ch in range(CH):
                        col = (
                            gat_s[:, ch, bb : bb + 1]
                            if t == 0
                            else gat_n[:, t - 1, ch, bb : bb + 1]
                        )
                        nc.vector.tensor_scalar_mul(
                            out=tm[:, ch, :],
                            in0=tiles2[t][:, bb, ch, :],
                            scalar1=col,
                        )
                    terms.append(tm)
                s1 = spool.tile([128, CH, HW], FP16, name="s1", tag="s1", bufs=2)
                nc.vector.tensor_tensor(
                    out=s1[:], in0=terms[0][:], in1=terms[1][:], op=ALU.add
                )
                s2 = spool.tile([128, CH, HW], FP16, name="s2", tag="s2", bufs=2)
                nc.gpsimd.tensor_tensor(
                    out=s2[:], in0=terms[2][:], in1=terms[3][:], op=ALU.add
                )
                s3 = spool.tile([128, CH, HW], FP16, name="s3", tag="s3", bufs=2)
                nc.vector.tensor_tensor(
                    out=s3[:], in0=s1[:], in1=terms[4][:], op=ALU.add
                )
                nc.vector.tensor_tensor(
                    out=acc2[:, bb], in0=s2[:], in1=s3[:], op=ALU.add
                )
            nc.scalar.dma_start(
                out=d_out[2 * pp : 2 * pp + 2].rearrange("b p c f -> p b c f"),
                in_=acc2[:],
            )

        for p in [i % NP for i in range(NP * repeat)]:
            # One 1 MiB DMA per tensor for the batch pair: [128, 2, CH, HW].
            tiles2 = []
            for t in range(NT):
                dt_ = dpool.tile(
                    [128, 2, CH, HW], FP16, name=f"d{t}", tag=f"d{t}", bufs=2
                )
                nc.sync.dma_start(
                    out=dt_[:],
                    in_=d_in[t][2 * p : 2 * p + 2].rearrange("b p c f -> p b c f"),
                )
                tiles2.append(dt_)
            acc2 = dpool.tile([128, 2, CH, HW], FP16, name="acc2", tag="acc2", bufs=2)

            # Sums for BOTH batches into one pair tile [128, NCH, 2].
            sums_p = spool.tile(
                [128, NCH, 2], FP32, name="sums_p", tag="sums_p", bufs=2
            )
            for bb in range(2):
                for t in range(NT):
                    for ch in range(CH):
                        j = t * CH + ch
                        if j < 4:  # 4 chunks on DVE, 6 on ACT (engine balance)
                            scr_v = spool.tile(
                                [128, HW], FP16, name="scr_v", tag="scr_v", bufs=2
                            )
                            nc.vector.tensor_scalar(
                                out=scr_v[:],
                                in0=tiles2[t][:, bb, ch, :],
                                scalar1=1.0,
                                scalar2=None,
                                op0=ALU.mult,
                                op1=ALU.add,
                                accum_out=sums_p[:, j, bb : bb + 1],
                            )
                        else:
                            scr_a = spool.tile(
                                [128, HW], FP16, name="scr_a", tag="scr_a", bufs=2
                            )
                            nc.scalar.activation(
                                out=scr_a[:],
                                in_=tiles2[t][:, bb, ch, :],
                                func=AF.Copy,
                                accum_out=sums_p[:, j, bb : bb + 1],
                            )

            # Gate MLP + activations, both batches as 2 moving columns.
            hps = ppool.tile([MID, 2], FP32, name="hps", tag="hps", bufs=2)
            for j in range(NCH):
                nc.tensor.matmul(
                    hps[:],
                    w1T[:, j, :],
                    sums_p[:, j, :],
                    start=(j == 0),
                    stop=(j == NCH - 1),
                )
            h_sb = spool.tile([MID, 2], FP32, name="h_sb", tag="h_sb", bufs=2)
            nc.scalar.activation(
                out=h_sb[:], in_=hps[:], func=AF.Relu,
                bias=bias_eff[:], scale=scale_eff[:],
            )
            wps = ppool.tile([128, NCH, 2], FP32, name="wps", tag="wps", bufs=2)
            for j in range(NCH):
                nc.tensor.matmul(
                    wps[:, j, :], w2T[:, j, :], h_sb[:], start=True, stop=True
                )
            lg = spool.tile([128, NCH, 2], FP32, name="lg", tag="lg", bufs=2)
            nc.vector.tensor_tensor(out=lg[:], in0=wps[:], in1=c2bT[:], op=ALU.add)
            gat_s = spool.tile([128, CH, 2], FP32, name="gat_s", tag="gat_s", bufs=2)
            nc.scalar.activation(out=gat_s[:], in_=lg[:, 0:CH, :], func=AF.Sigmoid)
            gat_e = spool.tile(
                [128, K * CH, 2], FP32, name="gat_e", tag="gat_e", bufs=2
            )
            nc.scalar.activation(out=gat_e[:], in_=lg[:, CH:NCH, :], func=AF.Exp)
            # softmax denominators over k for each (ch, bb).
            gk = gat_e[:].rearrange("p (k c) b -> p (c b) k", c=CH)
            esum = spool.tile([128, CH * 2, 1], FP32, name="esum", tag="esum", bufs=2)
            nc.vector.reduce_sum(out=esum[:], in_=gk, axis=mybir.AxisListType.X)
            rinv = spool.tile([128, CH * 2, 1], FP32, name="rinv", tag="rinv", bufs=2)
            nc.vector.reciprocal(rinv[:], esum[:])
            # Normalized branch gates gat_n[:, k, ch, bb].
            gat_n = spool.tile(
                [128, K, CH, 2], FP32, name="gat_n", tag="gat_n", bufs=2
            )
            ge_v = gat_e[:].rearrange("p (k c) b -> p k c b", c=CH)
            for ch in range(CH):
                for bb in range(2):
                    nc.vector.tensor_scalar_mul(
                        out=gat_n[:, :, ch, bb],
                        in0=ge_v[:, :, ch, bb],
                        scalar1=rinv[:, ch * 2 + bb, :],
                    )
            gates = ((gat_s, gat_n),)

            # Pass 2 + store for the PREVIOUS pair (its gates are ready).
            if prev is not None:
                emit_pass2(prev)
            prev = (tiles2, acc2, gates, p)

        if prev is not None:
            emit_pass2(prev)

    _split_waits(nc)
    return nc


_CACHE: dict = {}


def _get_program() -> bass.Bass:
    if "nc" not in _CACHE:
        _CACHE["nc"] = build_program()
    return _CACHE["nc"]


def make_in_maps(inputs: dict) -> list:
    """Shard full inputs into per-core input maps (batch-parallel)."""
    f32 = lambda a: np.asarray(a, dtype=np.float32)
    # [B, C, H, W] -> [B, 128, CH, HW] fp16, channel-on-partition (c = ch*128+p).
    def prep(a):
        a = f32(a).reshape(B, CH, 128, HW).transpose(0, 2, 1, 3)
        return np.ascontiguousarray(a, dtype=np.float16)

    y = prep(inputs["y"])
    xs = [prep(inputs[f"x{k}"]) for k in range(K)]

    conv1_w = f32(inputs["conv1_w"])
    conv2_w = f32(inputs["conv2_w"])
    gamma = f32(inputs["bn_gamma"])
    beta = f32(inputs["bn_beta"])
    mean = f32(inputs["bn_mean"])
    var = f32(inputs["bn_var"])
    s = gamma / np.sqrt(var + EPS)
    shared = {
        "w1T": np.ascontiguousarray(
            conv1_w.reshape(MID, NCH, 128).transpose(2, 1, 0)
        ),
        "w2T": np.ascontiguousarray(
            conv2_w.reshape(NCH, 128, MID).transpose(2, 0, 1)
        ),
        "c2bT": np.ascontiguousarray(
            np.repeat(f32(inputs["conv2_b"]).reshape(NCH, 128).T[:, :, None], 2, axis=2)
        ),
        "scale_eff": np.ascontiguousarray((s / HW).reshape(MID, 1)),
        "bias_eff": np.ascontiguousarray((beta - mean * s).reshape(MID, 1)),
    }
    in_maps = []
    for core in range(N_CORES):
        sl = slice(core * B_LOC, (core + 1) * B_LOC)
        m = {"y": np.ascontiguousarray(y[sl])}
        for k in range(K):
            m[f"x{k}"] = np.ascontiguousarray(xs[k][sl])
        m.update(shared)
        in_maps.append(m)
    return in_maps


def kernel(**inputs) -> np.ndarray:
    nc = _get_program()
    in_maps = make_in_maps(inputs)
    res = run_bass_kernel_spmd(nc, in_maps, list(range(N_CORES)))
    _CACHE["last_results"] = res
    # out [B_LOC, 128, CH, HW] fp16 -> [B_LOC, C, H, W] f32 (c = ch*128+p).
    out = np.concatenate(
        [
            np.asarray(res.results[i]["out"], dtype=np.float32)
            .transpose(0, 2, 1, 3)
            .reshape(B_LOC, C, H, W)
            for i in range(N_CORES)
        ],
        axis=0,
    )
    return out
